# revision 1
# baseline (speedup 1.0000x reference)
"""Trainium2 Bass kernel for nn_DeepseekLayer (dense transformer layer).

Sharding (8 cores): Megatron-style TP.
  - attention: head-sharded (2 heads/core); q/k/v projections over head shards;
    transposed-softmax layout (scores [sk, sq]); AllToAll switches attention
    output to token shards so o_proj needs no all-reduce.
  - o_proj + residual + rmsnorm2: token-sharded (256 tokens/core).
  - MLP: AllGather hidden -> tensor-parallel gate/up/down (1024 ff dims/core)
    -> chunked ReduceScatter (overlapped with down) -> local residual add ->
    host gathers token shards.
All heavy matmuls run in float32r (fp32 bits rounded to 11 mantissa bits,
1 PE cycle/row). Weights are pre-transposed/pre-tiled/pre-rounded on host.
rmsnorm1 is folded into rope tables / V-copy scales (scaling commutes with
the linear projections), so qkv runs directly on the raw (rounded) input.
"""
import numpy as np
from contextlib import ExitStack

from concourse import bacc
import concourse.tile as tile
import concourse.mybir as mybir
from concourse.bass_utils import run_bass_kernel_spmd

F32 = mybir.dt.float32
F32R = mybir.dt.float32r
AF = mybir.ActivationFunctionType
OP = mybir.AluOpType

H = 2048          # hidden
NH = 16           # heads
HD = 128          # head dim
MLP = 8192
S = 2048          # sequence
B = 1
EPS = 1e-6
NC = 8            # cores
HPC = NH // NC    # heads per core = 2
EH = HPC * HD     # qkv out dims per core = 256
MSH = MLP // NC   # mlp dims per core = 1024
SSH = S // NC     # tokens per shard = 256
RG = [list(range(NC))]
DT = H // 128     # 16 d-tiles
MT = MSH // 128   # 8 m-tiles per core


def round_fp32r(x: np.ndarray) -> np.ndarray:
    """Round fp32 to fp32r (11 mantissa bits, RNE) — matches walrus fp32_to_fp32r."""
    u = np.ascontiguousarray(x, dtype=np.float32).view(np.uint32).astype(np.uint64)
    r = (u + 0x7FF + ((u >> 12) & 1)) & 0xFFFFF000
    return r.astype(np.uint32).view(np.float32)


_LDW_PATCHED = False


def _enable_ldw_opt():
    """Compile our NEFF with walrus --enable-ldw-opt=true (elides redundant
    LDWEIGHTS; concourse's default is false)."""
    global _LDW_PATCHED
    if _LDW_PATCHED:
        return
    import concourse.bass_utils as _bu
    _orig = _bu.run_command

    def _patched(argv, **kw):
        argv = ["--enable-ldw-opt=true" if a == "--enable-ldw-opt=false" else a
                for a in argv]
        return _orig(argv, **kw)

    _bu.run_command = _patched
    _LDW_PATCHED = True


def _build_program():
    _enable_ldw_opt()
    nc = bacc.Bacc(trn_type="TRN2", target_bir_lowering=False, debug=False,
                   num_devices=NC)

    def inp(name, shape, dt):
        return nc.dram_tensor(name, shape, dt, kind="ExternalInput").ap()

    xTr = inp("xTr", [H, S], F32R)              # round_fp32r(x).T (feature-major)
    xTrs = inp("xTrs", [H, SSH], F32R)          # this core's token-shard of xTr
    cosT = inp("cosT", [HD, S], F32)
    sinT = inp("sinT", [HD, S], F32)
    wqT = inp("wqT", [H, EH], F32R)             # (wq*n1w/sqrt(HD)).T shard
    wkT = inp("wkT", [H, EH], F32R)             # (wk*n1w).T shard
    wvT = inp("wvT", [H, EH], F32R)             # (wv*n1w).T shard
    woTt = inp("woTt", [128, DT, DT, 128], F32R)   # wo.T tiled [p, et, dt, c]
    wgTt = inp("wgTt", [128, DT, MT, 128], F32R)   # (wg*n2w).T shard tiled [p, dt, mt, c]
    wuTt = inp("wuTt", [128, DT, MT, 128], F32R)
    wdTt = inp("wdTt", [128, MT, DT, 128], F32R)   # wd shard.T tiled [p, mt, dt, c]
    out_sh = nc.dram_tensor("out_sh", [H, SSH], F32, kind="ExternalOutput").ap()

    with tile.TileContext(nc) as tc, ExitStack() as top:
        dram = top.enter_context(tc.tile_pool(name="dram", bufs=1, space="DRAM"))
        per = top.enter_context(tc.tile_pool(name="per", bufs=1))
        ones_f = per.tile([128, 1], F32)
        nc.gpsimd.memset(ones_f[:], 1.0)
        ones_r = per.tile([128, 1], F32R)
        nc.vector.tensor_copy(ones_r[:], ones_f[:])
        ones_row = per.tile([1, 128], F32)
        nc.gpsimd.memset(ones_row[:], 1.0)
        one_s = per.tile([1, 1], F32)
        nc.gpsimd.memset(one_s[:], 1.0)
        eps1 = per.tile([1, 1], F32)
        nc.gpsimd.memset(eps1[:], EPS)
        from concourse.masks import make_identity
        ident_f = per.tile([128, 128], F32)
        make_identity(nc, ident_f[:])
        ident_r = per.tile([128, 128], F32R)
        nc.vector.tensor_copy(ident_r[:], ident_f[:])

        qk_ctx = ExitStack()
        qk_pool = qk_ctx.enter_context(tc.tile_pool(name="qk", bufs=1))
        qr = [qk_pool.tile([128, S], F32R, name=f"qr{h}") for h in range(HPC)]
        kr = [qk_pool.tile([128, S], F32R, name=f"kr{h}") for h in range(HPC)]
        V_sb = qk_pool.tile([128, S // 128, EH], F32R, name="V_sb")
        att = [qk_pool.tile([128, S], F32R, name=f"att{h}") for h in range(HPC)]

        s12_ctx = ExitStack()
        s12 = s12_ctx.enter_context(tc.tile_pool(name="s12", bufs=1))
        cos_sb = s12.tile([HD, S], F32, name="cos_sb")
        sin_sb = s12.tile([HD, S], F32, name="sin_sb")
        wq_sb = s12.tile([128, DT, EH], F32R, name="wq_sb")
        wk_sb = s12.tile([128, DT, EH], F32R, name="wk_sb")
        wv_sb = s12.tile([128, DT, EH], F32R, name="wv_sb")

        # ---- S12: fused rmsnorm1 stats + qkv + rope + V (per s-quarter) ----
        # norm scaling commutes with the projections:
        #   q_normed = rstd[s] * (x @ wq.T)  -> fold rstd into rope cos/sin
        #   v_normed = rstd[s] * v           -> fold rstd into the V PSUM copy
        with tc.tile_pool(name="s2", bufs=1) as s2, \
             tc.tile_pool(name="ps2", bufs=1, space="PSUM") as ps2:
            # pass 1+2 (s-halves): v projection + sumsq stats; pass 3+4: q/k.
            # xTr is streamed per pass; weights stay stationary across the two
            # 512-wide chunks of each half (1 LDWEIGHTS per 2 matmuls).
            rstd_bc = [None, None, None, None]
            for half in range(2):
                hb = half * 1024
                ss_ps = [ps2.tile([1, 512], F32, tag=f"pp{i}", name=f"ss{i}", bufs=1)
                         for i in range(2)]
                v_ps = [[ps2.tile([128, 512], F32, tag=f"pp{2 + h * 2 + i}",
                                  name=f"v{h}{i}", bufs=1)
                         for i in range(2)] for h in range(HPC)]
                for dt in range(DT):
                    xt = s2.tile([128, 1024], F32R, tag="xv", name="xv", bufs=3)
                    nc.sync.dma_start(xt[:], xTr[dt * 128:(dt + 1) * 128, hb:hb + 1024])
                    if half == 0:
                        nc.sync.dma_start(wv_sb[:, dt, :], wvT[dt * 128:(dt + 1) * 128, :])
                    x2 = s2.tile([128, 1024], F32R, tag="x2", bufs=2)
                    nc.vector.tensor_tensor(out=x2[:], in0=xt[:], in1=xt[:], op=OP.mult)
                    for i in range(2):
                        nc.tensor.matmul(ss_ps[i][:], ones_r[:],
                                         x2[:, i * 512:(i + 1) * 512],
                                         start=(dt == 0), stop=(dt == DT - 1))
                    for h in range(HPC):
                        for i in range(2):
                            nc.tensor.matmul(v_ps[h][i][:],
                                             wv_sb[:, dt, h * 128:(h + 1) * 128],
                                             xt[:, i * 512:(i + 1) * 512],
                                             start=(dt == 0), stop=(dt == DT - 1))
                # rstd for both quarters of this half
                for i in range(2):
                    qd = half * 2 + i
                    ssq = s2.tile([1, 512], F32, tag="ssq", bufs=2)
                    nc.scalar.activation(ssq[:], ss_ps[i][:], AF.Sqrt, bias=eps1[:],
                                         scale=1.0 / H)
                    rstd = s2.tile([1, 512], F32, tag="rstd", bufs=4)
                    nc.vector.reciprocal(rstd[:], ssq[:])
                    bc_ps = ps2.tile([128, 512], F32, tag=f"pp{i}", name="bc_ps", bufs=1)
                    nc.tensor.matmul(bc_ps[:], ones_row[:], rstd[:], start=True, stop=True)
                    rb = s2.tile([128, 512], F32, tag=f"rstd_bc{qd}", name=f"rb{qd}",
                                 bufs=1)
                    nc.vector.tensor_copy(rb[:], bc_ps[:])
                    rstd_bc[qd] = rb
                # V: scale vT by rstd, then PE-transpose to token-major V_sb
                for h in range(HPC):
                    for i in range(2):
                        qd = half * 2 + i
                        vsc = s2.tile([128, 512], F32R, tag="vsc", bufs=2)
                        nc.vector.tensor_tensor(out=vsc[:], in0=v_ps[h][i][:],
                                                in1=rstd_bc[qd][:], op=OP.mult)
                        for sti in range(4):
                            st = qd * 4 + sti
                            tr_ps = ps2.tile([128, 128], F32R, tag=f"pp{6 + h}",
                                             name="tr_ps", bufs=1)
                            nc.tensor.transpose(tr_ps[:],
                                                vsc[:, sti * 128:(sti + 1) * 128],
                                                ident_r[:])
                            nc.vector.tensor_copy(V_sb[:, st, h * 128:(h + 1) * 128],
                                                  tr_ps[:])
                if half == 0:
                    # overlap q/k weight loads + rope tables with pass-2 compute
                    for dt in range(DT):
                        nc.sync.dma_start(wq_sb[:, dt, :], wqT[dt * 128:(dt + 1) * 128, :])
                        nc.sync.dma_start(wk_sb[:, dt, :], wkT[dt * 128:(dt + 1) * 128, :])
                    nc.sync.dma_start(cos_sb[:], cosT)
                    nc.sync.dma_start(sin_sb[:], sinT)
            for half in range(2):
                hb = half * 1024
                q_ps = [[ps2.tile([128, 512], F32, tag=f"pp{h * 2 + i}",
                                  name=f"q{h}{i}", bufs=1)
                         for i in range(2)] for h in range(HPC)]
                k_ps = [[ps2.tile([128, 512], F32, tag=f"pp{4 + h * 2 + i}",
                                  name=f"k{h}{i}", bufs=1)
                         for i in range(2)] for h in range(HPC)]
                for dt in range(DT):
                    xt = s2.tile([128, 1024], F32R, tag="xv", name="xv2", bufs=3)
                    nc.sync.dma_start(xt[:], xTr[dt * 128:(dt + 1) * 128, hb:hb + 1024])
                    for h in range(HPC):
                        for i in range(2):
                            nc.tensor.matmul(q_ps[h][i][:],
                                             wq_sb[:, dt, h * 128:(h + 1) * 128],
                                             xt[:, i * 512:(i + 1) * 512],
                                             start=(dt == 0), stop=(dt == DT - 1))
                        for i in range(2):
                            nc.tensor.matmul(k_ps[h][i][:],
                                             wk_sb[:, dt, h * 128:(h + 1) * 128],
                                             xt[:, i * 512:(i + 1) * 512],
                                             start=(dt == 0), stop=(dt == DT - 1))
                for i in range(2):
                    qd = half * 2 + i
                    c0 = qd * 512
                    cs_c = s2.tile([HD, 512], F32, tag="cs_c", bufs=2)
                    nc.vector.tensor_tensor(out=cs_c[:], in0=cos_sb[:, c0:c0 + 512],
                                            in1=rstd_bc[qd][:], op=OP.mult)
                    cs_s = s2.tile([HD, 512], F32, tag="cs_s", bufs=2)
                    nc.vector.tensor_tensor(out=cs_s[:], in0=sin_sb[:, c0:c0 + 512],
                                            in1=rstd_bc[qd][:], op=OP.mult)
                    for h in range(HPC):
                        for (src_ps, dst) in ((q_ps[h][i], qr[h]), (k_ps[h][i], kr[h])):
                            m1 = s2.tile([64, 512], F32, tag="m1", bufs=1)
                            m2 = s2.tile([64, 512], F32, tag="m2", bufs=1)
                            nc.vector.tensor_tensor(out=m1[:], in0=src_ps[0:64, :],
                                                    in1=cs_c[0:64, :], op=OP.mult)
                            nc.vector.tensor_tensor(out=m2[:], in0=src_ps[64:128, :],
                                                    in1=cs_s[0:64, :], op=OP.mult)
                            nc.vector.tensor_tensor(out=dst[0:64, c0:c0 + 512],
                                                    in0=m1[:], in1=m2[:], op=OP.subtract)
                            m3 = s2.tile([64, 512], F32, tag="m3", bufs=1)
                            m4 = s2.tile([64, 512], F32, tag="m4", bufs=1)
                            nc.vector.tensor_tensor(out=m3[:], in0=src_ps[64:128, :],
                                                    in1=cs_c[64:128, :], op=OP.mult)
                            nc.vector.tensor_tensor(out=m4[:], in0=src_ps[0:64, :],
                                                    in1=cs_s[64:128, :], op=OP.mult)
                            nc.vector.tensor_tensor(out=dst[64:128, c0:c0 + 512],
                                                    in0=m3[:], in1=m4[:], op=OP.add)

        # ---- S3: attention (transposed softmax, no max subtraction) ----
        s12_ctx.close()
        a2a_in = [dram.tile([NC, 128, SSH], F32R, name=f"a2a_in{h}") for h in range(HPC)]
        a2a_out = [dram.tile([NC, 128, SSH], F32R, name=f"a2a_out{h}") for h in range(HPC)]
        with tc.tile_pool(name="s3", bufs=1) as s3, \
             tc.tile_pool(name="ps3", bufs=1, space="PSUM") as ps3:
            for h in range(HPC):
                for scp in range(2):     # pairs of sq-512 chunks (lhsT reuse x2)
                    q0 = scp * 1024
                    q1 = q0 + 512
                    av0 = ps3.tile([128, 512], F32, tag="av0", name="av0", bufs=1)
                    av1 = ps3.tile([128, 512], F32, tag="av1", name="av1", bufs=1)
                    sm0 = ps3.tile([1, 512], F32, tag="sm0", name="sm0", bufs=1)
                    sm1 = ps3.tile([1, 512], F32, tag="sm1", name="sm1", bufs=1)
                    for kt in range(DT):  # sk tiles of 128
                        st0 = ps3.tile([128, 512], F32, tag="st", name="st0", bufs=2)
                        nc.tensor.matmul(st0[:], kr[h][:, kt * 128:(kt + 1) * 128],
                                         qr[h][:, q0:q0 + 512], start=True, stop=True)
                        st1 = ps3.tile([128, 512], F32, tag="st", name="st1", bufs=2)
                        nc.tensor.matmul(st1[:], kr[h][:, kt * 128:(kt + 1) * 128],
                                         qr[h][:, q1:q1 + 512], start=True, stop=True)
                        e0 = s3.tile([128, 512], F32R, tag="e", bufs=4)
                        nc.scalar.activation(e0[:], st0[:], AF.Exp)
                        e1 = s3.tile([128, 512], F32R, tag="e", bufs=4)
                        nc.scalar.activation(e1[:], st1[:], AF.Exp)
                        nc.tensor.matmul(sm0[:], ones_r[:], e0[:],
                                         start=(kt == 0), stop=(kt == DT - 1))
                        nc.tensor.matmul(sm1[:], ones_r[:], e1[:],
                                         start=(kt == 0), stop=(kt == DT - 1))
                        nc.tensor.matmul(av0[:], V_sb[:, kt, h * 128:(h + 1) * 128],
                                         e0[:], start=(kt == 0), stop=(kt == DT - 1))
                        nc.tensor.matmul(av1[:], V_sb[:, kt, h * 128:(h + 1) * 128],
                                         e1[:], start=(kt == 0), stop=(kt == DT - 1))
                    for (qq, sm, av) in ((q0, sm0, av0), (q1, sm1, av1)):
                        rs_sb = s3.tile([1, 512], F32, tag="rs", bufs=2)
                        nc.vector.reciprocal(rs_sb[:], sm[:])
                        bc_ps = ps3.tile([128, 512], F32, tag="bc", name="bc_ps3", bufs=2)
                        nc.tensor.matmul(bc_ps[:], ones_row[:], rs_sb[:],
                                         start=True, stop=True)
                        bc_sb = s3.tile([128, 512], F32, tag="bcs", bufs=2)
                        nc.vector.tensor_copy(bc_sb[:], bc_ps[:])
                        nc.vector.tensor_tensor(out=att[h][:, qq:qq + 512], in0=av[:],
                                                in1=bc_sb[:], op=OP.mult)
                # ship this head's attention output while the next head computes
                for j in range(NC):
                    nc.sync.dma_start(a2a_in[h][j], att[h][:, j * SSH:(j + 1) * SSH])
                nc.gpsimd.collective_compute("AllToAll", OP.bypass,
                                             ins=[a2a_in[h][:]], outs=[a2a_out[h][:]],
                                             replica_groups=RG)

        # ---- S4: AllToAll (split per head) to token shards + o_proj + residual ----
        qk_ctx.close()
        res_pool = top.enter_context(tc.tile_pool(name="res", bufs=1))
        res1 = [res_pool.tile([128, SSH], F32, name=f"res1_{dt}") for dt in range(DT)]
        with tc.tile_pool(name="s4", bufs=1) as s4, \
             tc.tile_pool(name="ps4", bufs=1, space="PSUM") as ps4:
            attg = s4.tile([128, DT, SSH], F32R, tag="attg")
            for et in range(DT):
                nc.sync.dma_start(attg[:, et, :], a2a_out[et % 2][et // 2])
            for dt in range(DT):
                wo_t = s4.tile([128, DT, 128], F32R, tag="wo", bufs=2)
                nc.sync.dma_start(wo_t[:], woTt[:, :, dt, :])
                o_ps = ps4.tile([128, SSH], F32, tag="o", name="o_ps", bufs=2)
                for et in range(DT):
                    nc.tensor.matmul(o_ps[:], wo_t[:, et, :], attg[:, et, :],
                                     start=(et == 0), stop=(et == DT - 1))
                xs = s4.tile([128, SSH], F32R, tag="xs", bufs=2)
                nc.sync.dma_start(xs[:], xTrs[dt * 128:(dt + 1) * 128, :])
                nc.vector.tensor_tensor(out=res1[dt][:], in0=o_ps[:], in1=xs[:], op=OP.add)

        # ---- S5: rmsnorm2 on token shard ----
        h2_ctx = ExitStack()
        h2p = h2_ctx.enter_context(tc.tile_pool(name="h2p", bufs=1))
        h2 = [h2p.tile([128, SSH], F32R, name=f"h2_{dt}") for dt in range(DT)]
        with tc.tile_pool(name="s5", bufs=1) as s5, \
             tc.tile_pool(name="ps5", bufs=1, space="PSUM") as ps5:
            ss2_ps = ps5.tile([1, SSH], F32, tag="ss2", name="ss2_ps")
            for dt in range(DT):
                x2 = s5.tile([128, SSH], F32R, tag="x22", bufs=2)
                nc.vector.tensor_tensor(out=x2[:], in0=res1[dt][:], in1=res1[dt][:],
                                        op=OP.mult)
                nc.tensor.matmul(ss2_ps[:], ones_r[:], x2[:],
                                 start=(dt == 0), stop=(dt == DT - 1))
            ssq2 = s5.tile([1, SSH], F32, tag="ssq2")
            nc.scalar.activation(ssq2[:], ss2_ps[:], AF.Sqrt, bias=eps1[:], scale=1.0 / H)
            rstd2 = s5.tile([1, SSH], F32, tag="rstd2")
            nc.vector.reciprocal(rstd2[:], ssq2[:])
            bc2_ps = ps5.tile([128, SSH], F32, tag="bc2", name="bc2_ps", bufs=1)
            nc.tensor.matmul(bc2_ps[:], ones_row[:], rstd2[:], start=True, stop=True)
            rstd2_bc = s5.tile([128, SSH], F32, tag="rstd2bc")
            nc.vector.tensor_copy(rstd2_bc[:], bc2_ps[:])
            for dt in range(DT):
                nc.vector.tensor_tensor(out=h2[dt][:], in0=res1[dt][:],
                                        in1=rstd2_bc[:], op=OP.mult)

        # ---- S6: AllGather hidden shards (split in two d-halves) ----
        ag_in = [dram.tile([H // 2, SSH], F32R, name=f"ag_in{i}") for i in range(2)]
        ag_out = [dram.tile([NC, H // 2, SSH], F32R, addr_space="Shared",
                            name=f"ag_out{i}") for i in range(2)]
        for i in range(2):
            for k in range(DT // 2):
                dt = i * (DT // 2) + k
                nc.sync.dma_start(ag_in[i][k * 128:(k + 1) * 128, :], h2[dt][:])
            nc.gpsimd.collective_compute("AllGather", OP.bypass,
                                         ins=[ag_in[i][:]], outs=[ag_out[i][:]],
                                         replica_groups=RG)
        h2_ctx.close()

        # ---- S7: MLP gate/up (per s-half), then full-s down + chunked RS ----
        rs_in = [dram.tile([NC, 512, SSH], F32, name=f"rs_in{g}") for g in range(4)]
        rs_out = [dram.tile([512, SSH], F32, name=f"rs_out{g}") for g in range(4)]
        with tc.tile_pool(name="s7", bufs=1) as s7, \
             tc.tile_pool(name="ps7", bufs=1, space="PSUM") as ps7:
            act_t = [s7.tile([128, S], F32R, tag=f"act{mt}", name=f"act{mt}", bufs=1)
                     for mt in range(MT)]
            for half in range(2):        # s halves of 1024
                h2g = []
                for dt in range(DT):
                    t = s7.tile([128, 1024], F32R, tag=f"hg{dt}", name=f"hg{dt}", bufs=1)
                    gi, gr = (0, dt) if dt < DT // 2 else (1, dt - DT // 2)
                    for k in range(4):
                        r = half * 4 + k
                        nc.sync.dma_start(t[:, k * 256:(k + 1) * 256],
                                          ag_out[gi][r, gr * 128:(gr + 1) * 128, :])
                    h2g.append(t)
                hb = half * 1024
                for mt in range(MT):
                    wg_t = s7.tile([128, DT, 128], F32R, tag="wg", bufs=1)
                    wu_t = s7.tile([128, DT, 128], F32R, tag="wu", bufs=1)
                    nc.sync.dma_start(wg_t[:], wgTt[:, :, mt, :])
                    nc.sync.dma_start(wu_t[:], wuTt[:, :, mt, :])
                    g_ps = [ps7.tile([128, 512], F32, tag=f"g{i}", name=f"g{i}", bufs=1)
                            for i in range(2)]
                    u_ps = [ps7.tile([128, 512], F32, tag=f"u{i}", name=f"u{i}", bufs=1)
                            for i in range(2)]
                    for dt in range(DT):
                        nc.tensor.matmul(g_ps[0][:], wg_t[:, dt, :], h2g[dt][:, 0:512],
                                         start=(dt == 0), stop=(dt == DT - 1))
                        nc.tensor.matmul(g_ps[1][:], wg_t[:, dt, :], h2g[dt][:, 512:1024],
                                         start=(dt == 0), stop=(dt == DT - 1))
                    for dt in range(DT):
                        nc.tensor.matmul(u_ps[0][:], wu_t[:, dt, :], h2g[dt][:, 0:512],
                                         start=(dt == 0), stop=(dt == DT - 1))
                        nc.tensor.matmul(u_ps[1][:], wu_t[:, dt, :], h2g[dt][:, 512:1024],
                                         start=(dt == 0), stop=(dt == DT - 1))
                    for i in range(2):
                        gs = s7.tile([128, 512], F32, tag="gs", bufs=2)
                        nc.scalar.activation(gs[:], g_ps[i][:], AF.Sigmoid)
                        nc.vector.tensor_tensor(
                            out=act_t[mt][:, hb + i * 512:hb + (i + 1) * 512],
                            in0=u_ps[i][:], in1=gs[:], op=OP.mult)
            # down over full s, lhsT reused x4; RS issued per 4-dt group
            for grp in range(4):
                for dt in range(grp * 4, grp * 4 + 4):
                    wd_t = s7.tile([128, MT, 128], F32R, tag="wd", bufs=2)
                    nc.sync.dma_start(wd_t[:], wdTt[:, :, dt, :])
                    d_ps = [ps7.tile([128, 512], F32, tag=f"d{i}", name=f"d{i}", bufs=1)
                            for i in range(4)]
                    for mt in range(MT):
                        for i in range(4):
                            nc.tensor.matmul(d_ps[i][:], wd_t[:, mt, :],
                                             act_t[mt][:, i * 512:(i + 1) * 512],
                                             start=(mt == 0), stop=(mt == MT - 1))
                    for i in range(4):
                        dn = s7.tile([128, 512], F32, tag="dn", bufs=3)
                        nc.vector.tensor_copy(dn[:], d_ps[i][:])
                        dl = (dt - grp * 4) * 128
                        nc.sync.dma_start(rs_in[grp][2 * i, dl:dl + 128, :],
                                          dn[:, 0:256])
                        nc.sync.dma_start(rs_in[grp][2 * i + 1, dl:dl + 128, :],
                                          dn[:, 256:512])
                nc.gpsimd.collective_compute("ReduceScatter", OP.add,
                                             ins=[rs_in[grp][:]], outs=[rs_out[grp][:]],
                                             replica_groups=RG)

        with tc.tile_pool(name="s8", bufs=1) as s8:
            for grp in range(4):
                for k in range(4):
                    dt = grp * 4 + k
                    rsb = s8.tile([128, SSH], F32, tag="rsb", bufs=3)
                    nc.sync.dma_start(rsb[:], rs_out[grp][k * 128:(k + 1) * 128, :])
                    fin = s8.tile([128, SSH], F32, tag="fin", bufs=3)
                    nc.vector.tensor_tensor(out=fin[:], in0=rsb[:], in1=res1[dt][:],
                                            op=OP.add)
                    nc.sync.dma_start(out_sh[dt * 128:(dt + 1) * 128, :], fin[:])

    nc.compile()
    return nc


_PROG = None


def _get_program():
    global _PROG
    if _PROG is None:
        _PROG = _build_program()
    return _PROG


def _prep_inputs(x, norm1_w, wq, wk, wv, wo, norm2_w, w_gate, w_up, w_down, cos, sin):
    x = np.asarray(x, dtype=np.float32)
    xTr = round_fp32r(np.ascontiguousarray(x.reshape(S, H).T))         # [H, S]
    cosT = np.ascontiguousarray(np.asarray(cos, np.float32).T)         # [HD, S]
    sinT = np.ascontiguousarray(np.asarray(sin, np.float32).T)
    n1 = np.asarray(norm1_w, np.float32)
    n2 = np.asarray(norm2_w, np.float32)
    wq = np.asarray(wq, np.float32) * n1[None, :] / np.sqrt(np.float32(HD))
    wk = np.asarray(wk, np.float32) * n1[None, :]
    wv = np.asarray(wv, np.float32) * n1[None, :]
    wg = np.asarray(w_gate, np.float32) * n2[None, :]
    wu = np.asarray(w_up, np.float32) * n2[None, :]
    wo = np.asarray(wo, np.float32)
    wd = np.asarray(w_down, np.float32)

    woT = round_fp32r(wo.T)                                            # [e=H, d=H]
    woTt = np.ascontiguousarray(
        woT.reshape(DT, 128, DT, 128).transpose(1, 0, 2, 3))           # [p, et, dt, c]

    in_maps = []
    for c in range(NC):
        e0 = c * EH
        m0 = c * MSH
        wqT = round_fp32r(wq[e0:e0 + EH, :].T)                         # [H, EH]
        wkT = round_fp32r(wk[e0:e0 + EH, :].T)
        wvT = round_fp32r(wv[e0:e0 + EH, :].T)
        wgT = round_fp32r(wg[m0:m0 + MSH, :].T)                        # [H, MSH]
        wuT = round_fp32r(wu[m0:m0 + MSH, :].T)
        wdT = round_fp32r(wd[:, m0:m0 + MSH].T)                        # [MSH, H]
        in_maps.append({
            "xTr": xTr,
            "xTrs": np.ascontiguousarray(xTr[:, c * SSH:(c + 1) * SSH]),
            "cosT": cosT, "sinT": sinT,
            "wqT": np.ascontiguousarray(wqT),
            "wkT": np.ascontiguousarray(wkT),
            "wvT": np.ascontiguousarray(wvT),
            "woTt": woTt,
            "wgTt": np.ascontiguousarray(
                wgT.reshape(DT, 128, MT, 128).transpose(1, 0, 2, 3)),
            "wuTt": np.ascontiguousarray(
                wuT.reshape(DT, 128, MT, 128).transpose(1, 0, 2, 3)),
            "wdTt": np.ascontiguousarray(
                wdT.reshape(MT, 128, DT, 128).transpose(1, 0, 2, 3)),
        })
    return in_maps


def kernel(x, norm1_w, wq, wk, wv, wo, norm2_w, w_gate, w_up, w_down, cos, sin,
           _want_results=False):
    in_maps = _prep_inputs(x, norm1_w, wq, wk, wv, wo, norm2_w,
                           w_gate, w_up, w_down, cos, sin)
    prog = _get_program()
    res = run_bass_kernel_spmd(prog, in_maps, list(range(NC)))
    out = np.empty((B, S, H), dtype=np.float32)
    for c in range(NC):
        out[0, c * SSH:(c + 1) * SSH, :] = res.results[c]["out_sh"].T
    if _want_results:
        return out, res
    return out



# revision 14
# speedup vs baseline: 1.2144x; 1.2144x over previous
"""Trainium2 Bass kernel for nn_DeepseekLayer (dense transformer layer).

Sharding (8 cores): Megatron-style TP, all matmul operands bf16 (fp32 PSUM).
  - qkv: head-sharded (2 heads/core), single pass over x (streamed once, bf16),
    per-512-token-quarter PSUM accumulation; rmsnorm1 folded into rope tables /
    V scaling.
  - attention: transposed-softmax layout (scores [sk, sq]); per-head AllToAll
    (bf16) switches attention output to token shards.
  - o_proj + residual + rmsnorm2: token-sharded; head-0 tiles accumulate first
    so o_proj starts as soon as the first AllToAll lands; norm2 stats fold in.
  - MLP: AllGather hidden (bf16, 2 halves) -> gate/up/down with SBUF-resident
    weights, pipelined per 512-token chunk; per-chunk ReduceScatter over
    d-slices feeds a final small AllToAll that re-shards to tokens.
"""
import numpy as np
from contextlib import ExitStack

from concourse import bacc
import concourse.tile as tile
import concourse.mybir as mybir
from concourse.bass_utils import run_bass_kernel_spmd

F32 = mybir.dt.float32
F32R = mybir.dt.float32r
BF16 = mybir.dt.bfloat16
AF = mybir.ActivationFunctionType
OP = mybir.AluOpType

H = 2048          # hidden
NH = 16           # heads
HD = 128          # head dim
MLP = 8192
S = 2048          # sequence
B = 1
EPS = 1e-6
NC = 8            # cores
HPC = NH // NC    # heads per core = 2
EH = HPC * HD     # qkv out dims per core = 256
MSH = MLP // NC   # mlp dims per core = 1024
SSH = S // NC     # tokens per shard = 256
RG = [list(range(NC))]
DT = H // 128     # 16 d-tiles
MT = MSH // 128   # 8 m-tiles per core
SQ = 512          # s-chunk
NQ = S // SQ      # 4 chunks


_LDW_PATCHED = False


def _enable_ldw_opt():
    """Compile our NEFF with walrus --enable-ldw-opt=true (elides redundant
    LDWEIGHTS; concourse's default is false)."""
    global _LDW_PATCHED
    if _LDW_PATCHED:
        return
    import concourse.bass_utils as _bu
    _orig = _bu.run_command

    def _patched(argv, **kw):
        argv = ["--enable-ldw-opt=true" if a == "--enable-ldw-opt=false" else a
                for a in argv]
        return _orig(argv, **kw)

    _bu.run_command = _patched
    _LDW_PATCHED = True


def _build_program():
    nc = bacc.Bacc(trn_type="TRN2", target_bir_lowering=False, debug=False,
                   num_devices=NC)

    def inp(name, shape, dt):
        return nc.dram_tensor(name, shape, dt, kind="ExternalInput").ap()

    xTb = inp("xTb", [H, S], BF16)               # x.T bf16 (feature-major)
    xTrs = inp("xTrs", [H, SSH], F32)            # this core's token-shard of x.T
    cosT = inp("cosT", [HD, S], F32)
    sinT = inp("sinT", [HD, S], F32)
    wqT = inp("wqT", [H, EH], BF16)              # (wq*n1w/sqrt(HD)).T shard
    wkT = inp("wkT", [H, EH], BF16)              # (wk*n1w).T shard
    wvT = inp("wvT", [H, EH], BF16)              # (wv*n1w).T shard
    woTt = inp("woTt", [128, DT, DT, 128], BF16)    # wo.T tiled [p, et, dt, c]
    wgTt = inp("wgTt", [128, DT, MT, 128], BF16)    # (wg*n2w).T shard [p, dt, mt, c]
    wuTt = inp("wuTt", [128, DT, MT, 128], BF16)
    wdTt = inp("wdTt", [128, MT, DT, 128], BF16)    # wd shard.T tiled [p, mt, dt, c]
    out_sh = nc.dram_tensor("out_sh", [H, SSH], F32, kind="ExternalOutput").ap()

    with tile.TileContext(nc) as tc, ExitStack() as top:
        dram = top.enter_context(tc.tile_pool(name="dram", bufs=1, space="DRAM"))
        per = top.enter_context(tc.tile_pool(name="per", bufs=1))
        ones_f = per.tile([128, 1], F32)
        nc.gpsimd.memset(ones_f[:], 1.0)
        ones_bf = per.tile([128, 1], BF16)
        nc.vector.tensor_copy(ones_bf[:], ones_f[:])
        ones_row_f = per.tile([1, 128], F32)
        nc.gpsimd.memset(ones_row_f[:], 1.0)
        ones_row_r = per.tile([1, 128], F32R)
        nc.vector.tensor_copy(ones_row_r[:], ones_row_f[:])
        eps1 = per.tile([1, 1], F32)
        nc.gpsimd.memset(eps1[:], EPS)
        from concourse.masks import make_identity
        ident_f = per.tile([128, 128], F32)
        make_identity(nc, ident_f[:])
        ident_bf = per.tile([128, 128], BF16)
        nc.vector.tensor_copy(ident_bf[:], ident_f[:])

        # MLP weights pool: allocated up-front (outlives qk pool)
        mlpw = top.enter_context(tc.tile_pool(name="mlpw", bufs=1))
        wg_sb = mlpw.tile([128, DT, MT, 128], BF16, name="wg_sb")
        wu_sb = mlpw.tile([128, DT, MT, 128], BF16, name="wu_sb")
        wd_sb = mlpw.tile([128, MT, DT, 128], BF16, name="wd_sb")

        qk_ctx = ExitStack()
        qk_pool = qk_ctx.enter_context(tc.tile_pool(name="qk", bufs=1))
        qr = [qk_pool.tile([128, S], BF16, name=f"qr{h}") for h in range(HPC)]
        kr = [qk_pool.tile([128, S], BF16, name=f"kr{h}") for h in range(HPC)]
        V_sb = qk_pool.tile([128, S // 128, EH], BF16, name="V_sb")
        att = [qk_pool.tile([128, S], BF16, name=f"att{h}") for h in range(HPC)]

        s12_ctx = ExitStack()
        s12 = s12_ctx.enter_context(tc.tile_pool(name="s12", bufs=1))
        cos_sb = s12.tile([HD, S], F32, name="cos_sb")
        sin_sb = s12.tile([HD, S], F32, name="sin_sb")
        wq_sb = s12.tile([128, DT, EH], BF16, name="wq_sb")
        wk_sb = s12.tile([128, DT, EH], BF16, name="wk_sb")
        wv_sb = s12.tile([128, DT, EH], BF16, name="wv_sb")
        for dt in range(DT):
            nc.sync.dma_start(wq_sb[:, dt, :], wqT[dt * 128:(dt + 1) * 128, :])
            nc.sync.dma_start(wk_sb[:, dt, :], wkT[dt * 128:(dt + 1) * 128, :])
            nc.sync.dma_start(wv_sb[:, dt, :], wvT[dt * 128:(dt + 1) * 128, :])
        nc.sync.dma_start(cos_sb[:], cosT)
        nc.sync.dma_start(sin_sb[:], sinT)

        # ---- S12: single-pass qkv + rmsnorm1 stats + rope + V, per s-quarter ----
        # norm scaling commutes with the projections:
        #   q_normed = rstd[s] * (x @ wq.T)  -> fold rstd into rope cos/sin
        #   v_normed = rstd[s] * v           -> fold rstd into the V PSUM copy
        with tc.tile_pool(name="s2", bufs=1) as s2, \
             tc.tile_pool(name="ps2", bufs=1, space="PSUM") as ps2:
            for qd in range(NQ):
                c0 = qd * SQ
                q_ps = [ps2.tile([128, SQ], F32, tag=f"pp{h}", name=f"q{h}", bufs=1)
                        for h in range(HPC)]
                k_ps = [ps2.tile([128, SQ], F32, tag=f"pp{2 + h}", name=f"k{h}", bufs=1)
                        for h in range(HPC)]
                v_ps = [ps2.tile([128, SQ], F32, tag=f"pp{4 + h}", name=f"v{h}", bufs=1)
                        for h in range(HPC)]
                ss_ps = ps2.tile([1, SQ], F32, tag="pp6", name="ss", bufs=1)
                for dt in range(DT):
                    xt = s2.tile([128, SQ], BF16, tag="xv", name="xv", bufs=3)
                    nc.sync.dma_start(xt[:], xTb[dt * 128:(dt + 1) * 128, c0:c0 + SQ])
                    x2 = s2.tile([128, SQ], BF16, tag="x2", bufs=2)
                    nc.vector.tensor_tensor(out=x2[:], in0=xt[:], in1=xt[:], op=OP.mult)
                    nc.tensor.matmul(ss_ps[:], ones_bf[:], x2[:],
                                     start=(dt == 0), stop=(dt == DT - 1))
                    for h in range(HPC):
                        nc.tensor.matmul(v_ps[h][:], wv_sb[:, dt, h * 128:(h + 1) * 128],
                                         xt[:], start=(dt == 0), stop=(dt == DT - 1))
                    for h in range(HPC):
                        nc.tensor.matmul(q_ps[h][:], wq_sb[:, dt, h * 128:(h + 1) * 128],
                                         xt[:], start=(dt == 0), stop=(dt == DT - 1))
                        nc.tensor.matmul(k_ps[h][:], wk_sb[:, dt, h * 128:(h + 1) * 128],
                                         xt[:], start=(dt == 0), stop=(dt == DT - 1))
                # rstd for this quarter
                ssq = s2.tile([1, SQ], F32, tag="ssq", bufs=2)
                nc.scalar.activation(ssq[:], ss_ps[:], AF.Sqrt, bias=eps1[:],
                                     scale=1.0 / H)
                rstd = s2.tile([1, SQ], F32R, tag="rstd", bufs=2)
                with nc.allow_low_precision(reason="f32r holds full fp32 bits"):
                    nc.vector.reciprocal(rstd[:], ssq[:])
                bc_ps = ps2.tile([128, SQ], F32, tag="pp6", name="bc_ps", bufs=1)
                nc.tensor.matmul(bc_ps[:], ones_row_r[:], rstd[:], start=True, stop=True)
                rb = s2.tile([128, SQ], F32, tag="rb", bufs=2)
                nc.vector.tensor_copy(rb[:], bc_ps[:])
                # V: scale vT by rstd, then PE-transpose to token-major V_sb
                for h in range(HPC):
                    vsc = s2.tile([128, SQ], BF16, tag="vsc", bufs=2)
                    nc.vector.tensor_tensor(out=vsc[:], in0=v_ps[h][:], in1=rb[:],
                                            op=OP.mult)
                    for sti in range(SQ // 128):
                        st = qd * (SQ // 128) + sti
                        tr_ps = ps2.tile([128, 128], BF16, tag="pp7", name="tr_ps",
                                         bufs=1)
                        nc.tensor.transpose(tr_ps[:], vsc[:, sti * 128:(sti + 1) * 128],
                                            ident_bf[:])
                        nc.vector.tensor_copy(V_sb[:, st, h * 128:(h + 1) * 128],
                                              tr_ps[:])
                # rope on q/k with rstd folded into the tables
                cs_c = s2.tile([HD, SQ], F32, tag="cs_c", bufs=2)
                nc.vector.tensor_tensor(out=cs_c[:], in0=cos_sb[:, c0:c0 + SQ],
                                        in1=rb[:], op=OP.mult)
                cs_s = s2.tile([HD, SQ], F32, tag="cs_s", bufs=2)
                nc.vector.tensor_tensor(out=cs_s[:], in0=sin_sb[:, c0:c0 + SQ],
                                        in1=rb[:], op=OP.mult)
                for h in range(HPC):
                    for (src_ps, dst) in ((q_ps[h], qr[h]), (k_ps[h], kr[h])):
                        m1 = s2.tile([64, SQ], F32, tag="m1", bufs=1)
                        m2 = s2.tile([64, SQ], F32, tag="m2", bufs=1)
                        nc.vector.tensor_tensor(out=m1[:], in0=src_ps[0:64, :],
                                                in1=cs_c[0:64, :], op=OP.mult)
                        nc.vector.tensor_tensor(out=m2[:], in0=src_ps[64:128, :],
                                                in1=cs_s[0:64, :], op=OP.mult)
                        nc.vector.tensor_tensor(out=dst[0:64, c0:c0 + SQ],
                                                in0=m1[:], in1=m2[:], op=OP.subtract)
                        m3 = s2.tile([64, SQ], F32, tag="m3", bufs=1)
                        m4 = s2.tile([64, SQ], F32, tag="m4", bufs=1)
                        nc.vector.tensor_tensor(out=m3[:], in0=src_ps[64:128, :],
                                                in1=cs_c[64:128, :], op=OP.mult)
                        nc.vector.tensor_tensor(out=m4[:], in0=src_ps[0:64, :],
                                                in1=cs_s[64:128, :], op=OP.mult)
                        nc.vector.tensor_tensor(out=dst[64:128, c0:c0 + SQ],
                                                in0=m3[:], in1=m4[:], op=OP.add)
        s12_ctx.close()

        # MLP weights: DMA'd during attention (deps allow early issue)
        for dt in range(DT):
            nc.scalar.dma_start(wg_sb[:, dt, :, :], wgTt[:, dt, :, :])
            nc.scalar.dma_start(wu_sb[:, dt, :, :], wuTt[:, dt, :, :])
        for mt in range(MT):
            nc.scalar.dma_start(wd_sb[:, mt, :, :], wdTt[:, mt, :, :])

        # ---- S3: attention (transposed softmax, no max subtraction) ----
        a2a_in = [dram.tile([NC, 128, SSH], BF16, name=f"a2a_in{h}")
                  for h in range(HPC)]
        a2a_out = [dram.tile([NC, 128, SSH], BF16, name=f"a2a_out{h}")
                   for h in range(HPC)]
        with tc.tile_pool(name="s3", bufs=1) as s3, \
             tc.tile_pool(name="ps3", bufs=1, space="PSUM") as ps3:
            for h in range(HPC):
                for scp in range(2):     # pairs of sq-512 chunks (lhsT reuse x2)
                    q0 = scp * 1024
                    q1 = q0 + 512
                    av0 = ps3.tile([128, 512], F32, tag="av0", name="av0", bufs=1)
                    av1 = ps3.tile([128, 512], F32, tag="av1", name="av1", bufs=1)
                    sm0 = ps3.tile([1, 512], F32, tag="sm0", name="sm0", bufs=1)
                    sm1 = ps3.tile([1, 512], F32, tag="sm1", name="sm1", bufs=1)
                    for kt in range(DT):  # sk tiles of 128
                        st0 = ps3.tile([128, 512], F32, tag="st", name="st0", bufs=2)
                        nc.tensor.matmul(st0[:], kr[h][:, kt * 128:(kt + 1) * 128],
                                         qr[h][:, q0:q0 + 512], start=True, stop=True)
                        st1 = ps3.tile([128, 512], F32, tag="st", name="st1", bufs=2)
                        nc.tensor.matmul(st1[:], kr[h][:, kt * 128:(kt + 1) * 128],
                                         qr[h][:, q1:q1 + 512], start=True, stop=True)
                        e0 = s3.tile([128, 512], BF16, tag="e", bufs=4)
                        nc.scalar.activation(e0[:], st0[:], AF.Exp)
                        e1 = s3.tile([128, 512], BF16, tag="e", bufs=4)
                        nc.scalar.activation(e1[:], st1[:], AF.Exp)
                        nc.tensor.matmul(sm0[:], ones_bf[:], e0[:],
                                         start=(kt == 0), stop=(kt == DT - 1))
                        nc.tensor.matmul(sm1[:], ones_bf[:], e1[:],
                                         start=(kt == 0), stop=(kt == DT - 1))
                        nc.tensor.matmul(av0[:], V_sb[:, kt, h * 128:(h + 1) * 128],
                                         e0[:], start=(kt == 0), stop=(kt == DT - 1))
                        nc.tensor.matmul(av1[:], V_sb[:, kt, h * 128:(h + 1) * 128],
                                         e1[:], start=(kt == 0), stop=(kt == DT - 1))
                    for (qq, sm, av) in ((q0, sm0, av0), (q1, sm1, av1)):
                        rs_sb = s3.tile([1, 512], F32R, tag="rs", bufs=2)
                        with nc.allow_low_precision(reason="f32r holds full fp32 bits"):
                            nc.vector.reciprocal(rs_sb[:], sm[:])
                        bc_ps = ps3.tile([128, 512], F32, tag="bc", name="bc_ps3",
                                         bufs=2)
                        nc.tensor.matmul(bc_ps[:], ones_row_r[:], rs_sb[:],
                                         start=True, stop=True)
                        bc_sb = s3.tile([128, 512], F32, tag="bcs", bufs=2)
                        nc.vector.tensor_copy(bc_sb[:], bc_ps[:])
                        nc.vector.tensor_tensor(out=att[h][:, qq:qq + 512], in0=av[:],
                                                in1=bc_sb[:], op=OP.mult)
                # ship this head's attention output while the next head computes
                for j in range(NC):
                    nc.sync.dma_start(a2a_in[h][j], att[h][:, j * SSH:(j + 1) * SSH])
                nc.gpsimd.collective_compute("AllToAll", OP.bypass,
                                             ins=[a2a_in[h][:]], outs=[a2a_out[h][:]],
                                             replica_groups=RG)

        # ---- S4: o_proj on token shard (+ rmsnorm2 stats folded in) ----
        qk_ctx.close()
        res_pool = top.enter_context(tc.tile_pool(name="res", bufs=1))
        res1 = [res_pool.tile([128, SSH], F32, name=f"res1_{dt}") for dt in range(DT)]
        h2_ctx = ExitStack()
        h2p = h2_ctx.enter_context(tc.tile_pool(name="h2p", bufs=1))
        h2 = [h2p.tile([128, SSH], BF16, name=f"h2_{dt}") for dt in range(DT)]
        ag_in = [dram.tile([H // 2, SSH], BF16, name=f"ag_in{i}") for i in range(2)]
        ag_out = [dram.tile([NC, H // 2, SSH], BF16, addr_space="Shared",
                            name=f"ag_out{i}") for i in range(2)]
        # o accumulation order: head-0 tiles (even et) first so compute can
        # start as soon as the first AllToAll completes
        ET_ORDER = list(range(0, DT, 2)) + list(range(1, DT, 2))
        with tc.tile_pool(name="s4", bufs=1) as s4, \
             tc.tile_pool(name="ps4", bufs=1, space="PSUM") as ps4:
            attg = s4.tile([128, DT, SSH], BF16, tag="attg")
            for h in range(HPC):
                nc.sync.dma_start(attg[:, h::2, :], a2a_out[h][:].transpose([1, 0, 2]))
            xs_all = s4.tile([128, DT, SSH], F32, tag="xs")
            nc.sync.dma_start(xs_all[:], xTrs[:].rearrange("(d p) s -> p d s", p=128))
            ss2_ps = ps4.tile([1, SSH], F32, tag="ss2", name="ss2_ps")
            for dt in range(DT):
                wo_t = s4.tile([128, DT, 128], BF16, tag="wo", bufs=2)
                nc.sync.dma_start(wo_t[:], woTt[:, :, dt, :])
                o_ps = ps4.tile([128, SSH], F32, tag="o", name="o_ps", bufs=2)
                for i, et in enumerate(ET_ORDER):
                    nc.tensor.matmul(o_ps[:], wo_t[:, et, :], attg[:, et, :],
                                     start=(i == 0), stop=(i == DT - 1))
                nc.vector.tensor_tensor(out=res1[dt][:], in0=o_ps[:],
                                        in1=xs_all[:, dt, :], op=OP.add)
                x2 = s4.tile([128, SSH], BF16, tag="x22", bufs=2)
                nc.vector.tensor_tensor(out=x2[:], in0=res1[dt][:], in1=res1[dt][:],
                                        op=OP.mult)
                nc.tensor.matmul(ss2_ps[:], ones_bf[:], x2[:],
                                 start=(dt == 0), stop=(dt == DT - 1))
            # rmsnorm2 scale
            ssq2 = s4.tile([1, SSH], F32, tag="ssq2")
            nc.scalar.activation(ssq2[:], ss2_ps[:], AF.Sqrt, bias=eps1[:],
                                 scale=1.0 / H)
            rstd2 = s4.tile([1, SSH], F32R, tag="rstd2")
            with nc.allow_low_precision(reason="f32r holds full fp32 bits"):
                nc.vector.reciprocal(rstd2[:], ssq2[:])
            bc2_ps = ps4.tile([128, SSH], F32, tag="bc2", name="bc2_ps", bufs=1)
            nc.tensor.matmul(bc2_ps[:], ones_row_r[:], rstd2[:], start=True, stop=True)
            rstd2_bc = s4.tile([128, SSH], F32, tag="rstd2bc")
            nc.vector.tensor_copy(rstd2_bc[:], bc2_ps[:])
            for i in range(2):
                for k in range(DT // 2):
                    dt = i * (DT // 2) + k
                    nc.vector.tensor_tensor(out=h2[dt][:], in0=res1[dt][:],
                                            in1=rstd2_bc[:], op=OP.mult)
                    nc.sync.dma_start(ag_in[i][k * 128:(k + 1) * 128, :], h2[dt][:])
                nc.gpsimd.collective_compute("AllGather", OP.bypass,
                                             ins=[ag_in[i][:]], outs=[ag_out[i][:]],
                                             replica_groups=RG)
        h2_ctx.close()

        # ---- S7: MLP, pipelined per 512-token chunk with d-sliced RS ----
        # RS slot r = d-rows [r*256,(r+1)*256), content [2 tok-subshard, 256 d,
        # 256 s]; RS output writes straight into the final AllToAll's input.
        a2a_mlp_in = dram.tile([NC, 256, SSH], BF16, name="a2a_mlp_in")
        a2a_mlp_out = dram.tile([NC, 256, SSH], BF16, name="a2a_mlp_out")
        rs_in = [dram.tile([NC, 2, 256, SSH], BF16, name=f"rs_in{c}")
                 for c in range(NQ)]
        with tc.tile_pool(name="s7", bufs=1) as s7, \
             tc.tile_pool(name="ps7", bufs=1, space="PSUM") as ps7:
            act_t = [s7.tile([128, S], BF16, tag=f"act{mt}", name=f"act{mt}", bufs=1)
                     for mt in range(MT)]
            for c in range(NQ):
                c0 = c * SQ
                h2g = s7.tile([128, DT, SQ], BF16, tag="h2g", bufs=2)
                for dt in range(DT):
                    gi, gr = (0, dt) if dt < DT // 2 else (1, dt - DT // 2)
                    nc.sync.dma_start(
                        h2g[:, dt, :].rearrange("p (j s) -> p j s", j=2),
                        ag_out[gi][2 * c:2 * c + 2, gr * 128:(gr + 1) * 128, :]
                        .transpose([1, 0, 2]))
                for mt in range(MT):
                    g_ps = ps7.tile([128, SQ], F32, tag="g", name="g_ps", bufs=2)
                    u_ps = ps7.tile([128, SQ], F32, tag="u", name="u_ps", bufs=2)
                    for dt in range(DT):
                        nc.tensor.matmul(g_ps[:], wg_sb[:, dt, mt, :], h2g[:, dt, :],
                                         start=(dt == 0), stop=(dt == DT - 1))
                    for dt in range(DT):
                        nc.tensor.matmul(u_ps[:], wu_sb[:, dt, mt, :], h2g[:, dt, :],
                                         start=(dt == 0), stop=(dt == DT - 1))
                    gs = s7.tile([128, SQ], BF16, tag="gs", bufs=2)
                    nc.scalar.activation(gs[:], g_ps[:], AF.Sigmoid)
                    nc.vector.tensor_tensor(out=act_t[mt][:, c0:c0 + SQ],
                                            in0=u_ps[:], in1=gs[:], op=OP.mult)
                for r in range(NC):
                    dn_pair = s7.tile([128, 2, SQ], BF16, tag="dn", bufs=2)
                    for dtl in range(2):
                        dt = 2 * r + dtl
                        d_ps = ps7.tile([128, SQ], F32, tag="d", name="d_ps", bufs=2)
                        for mt in range(MT):
                            nc.tensor.matmul(d_ps[:], wd_sb[:, mt, dt, :],
                                             act_t[mt][:, c0:c0 + SQ],
                                             start=(mt == 0), stop=(mt == MT - 1))
                        nc.vector.tensor_copy(dn_pair[:, dtl, :], d_ps[:])
                    # dst slot r: [j, dtl*128+p, s] <- dn_pair[p, dtl, j*256+s]
                    for j in range(2):
                        nc.sync.dma_start(
                            rs_in[c][r][j].rearrange("(dtl p) s -> p dtl s", p=128),
                            dn_pair[:, :, j * SSH:(j + 1) * SSH])
                nc.gpsimd.collective_compute(
                    "ReduceScatter", OP.add, ins=[rs_in[c][:]],
                    outs=[a2a_mlp_in[2 * c:2 * c + 2, :, :]], replica_groups=RG)
            nc.gpsimd.collective_compute("AllToAll", OP.bypass,
                                         ins=[a2a_mlp_in[:]], outs=[a2a_mlp_out[:]],
                                         replica_groups=RG)

        with tc.tile_pool(name="s8", bufs=1) as s8:
            for r in range(NC):
                for sub in range(2):
                    dt = 2 * r + sub
                    rsb = s8.tile([128, SSH], BF16, tag="rsb", bufs=3)
                    nc.sync.dma_start(rsb[:], a2a_mlp_out[r, sub * 128:(sub + 1) * 128, :])
                    fin = s8.tile([128, SSH], F32, tag="fin", bufs=3)
                    nc.vector.tensor_tensor(out=fin[:], in0=rsb[:], in1=res1[dt][:],
                                            op=OP.add)
                    nc.sync.dma_start(out_sh[dt * 128:(dt + 1) * 128, :], fin[:])

    nc.compile()
    return nc


_PROG = None


def _get_program():
    global _PROG
    if _PROG is None:
        _PROG = _build_program()
    return _PROG


def _prep_inputs(x, norm1_w, wq, wk, wv, wo, norm2_w, w_gate, w_up, w_down, cos, sin):
    import ml_dtypes
    BF = ml_dtypes.bfloat16

    x = np.asarray(x, dtype=np.float32)
    xT = np.ascontiguousarray(x.reshape(S, H).T)                       # [H, S]
    xTb = xT.astype(BF)
    cosT = np.ascontiguousarray(np.asarray(cos, np.float32).T)         # [HD, S]
    sinT = np.ascontiguousarray(np.asarray(sin, np.float32).T)
    n1 = np.asarray(norm1_w, np.float32)
    n2 = np.asarray(norm2_w, np.float32)
    wq = np.asarray(wq, np.float32) * n1[None, :] / np.sqrt(np.float32(HD))
    wk = np.asarray(wk, np.float32) * n1[None, :]
    wv = np.asarray(wv, np.float32) * n1[None, :]
    wg = np.asarray(w_gate, np.float32) * n2[None, :]
    wu = np.asarray(w_up, np.float32) * n2[None, :]
    wo = np.asarray(wo, np.float32)
    wd = np.asarray(w_down, np.float32)

    woT = wo.T.astype(BF)                                              # [e=H, d=H]
    woTt = np.ascontiguousarray(
        woT.reshape(DT, 128, DT, 128).transpose(1, 0, 2, 3))           # [p, et, dt, c]

    in_maps = []
    for c in range(NC):
        e0 = c * EH
        m0 = c * MSH
        wqT = wq[e0:e0 + EH, :].T.astype(BF)                           # [H, EH]
        wkT = wk[e0:e0 + EH, :].T.astype(BF)
        wvT = wv[e0:e0 + EH, :].T.astype(BF)
        wgT = wg[m0:m0 + MSH, :].T.astype(BF)                          # [H, MSH]
        wuT = wu[m0:m0 + MSH, :].T.astype(BF)
        wdT = wd[:, m0:m0 + MSH].T.astype(BF)                          # [MSH, H]
        in_maps.append({
            "xTb": xTb,
            "xTrs": np.ascontiguousarray(xT[:, c * SSH:(c + 1) * SSH]),
            "cosT": cosT, "sinT": sinT,
            "wqT": np.ascontiguousarray(wqT),
            "wkT": np.ascontiguousarray(wkT),
            "wvT": np.ascontiguousarray(wvT),
            "woTt": woTt,
            "wgTt": np.ascontiguousarray(
                wgT.reshape(DT, 128, MT, 128).transpose(1, 0, 2, 3)),
            "wuTt": np.ascontiguousarray(
                wuT.reshape(DT, 128, MT, 128).transpose(1, 0, 2, 3)),
            "wdTt": np.ascontiguousarray(
                wdT.reshape(MT, 128, DT, 128).transpose(1, 0, 2, 3)),
        })
    return in_maps


def kernel(x, norm1_w, wq, wk, wv, wo, norm2_w, w_gate, w_up, w_down, cos, sin,
           _want_results=False):
    in_maps = _prep_inputs(x, norm1_w, wq, wk, wv, wo, norm2_w,
                           w_gate, w_up, w_down, cos, sin)
    prog = _get_program()
    res = run_bass_kernel_spmd(prog, in_maps, list(range(NC)))
    out = np.empty((B, S, H), dtype=np.float32)
    for c in range(NC):
        out[0, c * SSH:(c + 1) * SSH, :] = res.results[c]["out_sh"].T
    if _want_results:
        return out, res
    return out
